# revision 1
# baseline (speedup 1.0000x reference)
"""Trainium2 Bass kernel for nn_MemoryNetwork (scatter_memory).

Computation (reference, per batch row b):
    f = feature / ||feature||                       [B, 768]
    topic = f @ W_topic.T ; dom = f @ W_domain.T    [B, 256]
    att   = softmax_m(TAU * topic . memory[d,m])    [B, 9, 10]
    sep   = sum_m att * memory[d,m]                 [B, 9, 256]
    out   = softmax_d(TAU * sep . dom)              [B, 1, 9]

Reformulation: the memory banks are tiny, so fold them into the projection
weights on the host:
    P = mem_flat @ W_topic ; Q = mem_flat @ W_domain ; R = [P; Q]  [180, 768]
Per row only one [768 x 180] product is needed:
    raw    = feature @ R.T                   (rawS | rawT)
    r      = TAU / ||feature||
    ex     = exp(rawS * r - SHIFT)           (softmax_m numerator, const shift
                                              instead of max-subtraction; safe:
                                              logits are in [-130, 110])
    sums_d = sum_m ex ; wsum_d = sum_m ex * rawT
    datt   = (wsum / sums) * r               (= TAU * domain_att)
    out    = softmax_d(datt)                 (const shift again)

Precision/speed: the PE cannot amortize fp32 weight loads (each fp32 matmul
self-loads its stationary twice at ~260ns), so fp32 matmuls measure ~3x
slower than their streaming cost. Instead the matmul runs as a compensated
fp16 pair: f = fhi + flo, R = Rhi + Rlo (exact fp16 splits, done host-side),
raw = fhi@Rhi + fhi@Rlo + flo@Rhi accumulated in fp32 PSUM -- ~20-bit
effective mantissa, measured ~2e-4 absmax output error vs the fp32
reference. Same DMA bytes as fp32 (2 x fp16 planes).

Sharding: data-parallel over B across 8 cores (4096 rows each). Features are
sent transposed [768, 4096] so matmuls contract over partitions directly;
row norms (r = TAU/||f||) ride along from the same host pass.
"""

import sys

sys.path.insert(0, "/opt/trn_rl_repo")

import numpy as np

B, IN, E, D, M = 32768, 768, 256, 9, 10
NCORES = 8
BC = B // NCORES  # rows per core
P = 128           # partition tile
NT = BC // P      # batch tiles per core (32)
G = 8             # tiles per softmax group
NG = NT // G
DM = 2 * D * M    # 180
KC = IN // P      # contraction chunks (6)
TAU = 32.0
SHIFT = 50.0

_CACHE: dict = {}


def _build_nc(repeat=1):
    from contextlib import ExitStack

    import concourse.bacc as bacc
    import concourse.tile as tile
    from concourse import mybir

    F32 = mybir.dt.float32
    F16 = mybir.dt.float16
    AF = mybir.ActivationFunctionType

    nc = bacc.Bacc(trn_type="TRN2")
    fhi = nc.dram_tensor("fhi", [IN, BC], F16, kind="ExternalInput")
    flo = nc.dram_tensor("flo", [IN, BC], F16, kind="ExternalInput")
    # rt2[k] columns 0:180 = Rhi[k], 180:360 = Rlo[k]
    rt2 = nc.dram_tensor("rt2", [IN, 2 * DM], F16, kind="ExternalInput")
    rin = nc.dram_tensor("rin", [P, NT], F32, kind="ExternalInput")
    out = nc.dram_tensor("out", [BC, D], F32, kind="ExternalOutput")

    LB = 4 * P  # feature DMA block: 4 batch tiles per transfer
    with tile.TileContext(nc) as tc, ExitStack() as ctx:
        const = ctx.enter_context(tc.tile_pool(name="const", bufs=1))
        fpool = ctx.enter_context(tc.tile_pool(name="fts", bufs=4))
        rawpool = ctx.enter_context(tc.tile_pool(name="raws", bufs=4))
        gpool = ctx.enter_context(tc.tile_pool(name="grp", bufs=2))
        spool = ctx.enter_context(tc.tile_pool(name="small", bufs=2))
        raw_ps = ctx.enter_context(tc.tile_pool(name="rawps", bufs=6, space="PSUM"))

        # Constants (off the sync queue so the first feature block leads it)
        rt_sb = const.tile([P, KC, 2 * DM], F16)
        nc.scalar.dma_start(rt_sb[:], rt2[:, :].rearrange("(k p) j -> p k j", p=P))
        r_all = const.tile([P, NT], F32)
        nc.scalar.dma_start(r_all[:], rin[:, :])
        bias_shift = const.tile([P, 1], F32)
        nc.gpsimd.memset(bias_shift[:], -SHIFT)
        out_sb = const.tile([P, NT, D], F32)

        fhi_v = fhi[:, :].rearrange("(k p) b -> p k b", p=P)
        flo_v = flo[:, :].rearrange("(k p) b -> p k b", p=P)

        for g in range(NG * repeat):
            g = g % NG
            ex_g = gpool.tile([P, G, D * M], F32, tag="exg")
            t_g = gpool.tile([P, G, D * M], F32, tag="tg")

            # Loads: 4-tile blocks, alternating DMA issuers. The first group
            # uses single-tile blocks so the first matmul starts ~4x sooner.
            lb = P if g == 0 else LB
            hi_blocks, lo_blocks = [], []
            for h in range(G * P // lb):
                t0 = g * G * P + h * lb
                hi_sb = fpool.tile([P, KC, lb], F16, tag=f"fhi{min(g,1)}")
                lo_sb = fpool.tile([P, KC, lb], F16, tag=f"flo{min(g,1)}")
                eng_a = nc.sync if h % 2 == 0 else nc.gpsimd
                eng_b = nc.gpsimd if h % 2 == 0 else nc.sync
                eng_a.dma_start(hi_sb[:], fhi_v[:, :, t0 : t0 + lb])
                eng_b.dma_start(lo_sb[:], flo_v[:, :, t0 : t0 + lb])
                hi_blocks.append(hi_sb)
                lo_blocks.append(lo_sb)

            for s in range(G):
                t = g * G + s
                blk = s * P // lb
                sl = slice((s % (lb // P)) * P, (s % (lb // P) + 1) * P)
                hi_sb, lo_sb = hi_blocks[blk], lo_blocks[blk]
                raw = raw_ps.tile([P, DM], F32, tag="raw")
                for k in range(KC):
                    # raw += fhi@Rhi + fhi@Rlo + flo@Rhi  (all into one bank)
                    nc.tensor.matmul(
                        raw[:], hi_sb[:, k, sl], rt_sb[:, k, 0:DM],
                        start=(k == 0), stop=False,
                    )
                    nc.tensor.matmul(
                        raw[:], hi_sb[:, k, sl], rt_sb[:, k, DM : 2 * DM],
                        start=False, stop=False,
                    )
                    nc.tensor.matmul(
                        raw[:], lo_sb[:, k, sl], rt_sb[:, k, 0:DM],
                        start=False, stop=(k == KC - 1),
                    )
                nc.scalar.activation(
                    ex_g[:, s, :],
                    raw[:, 0 : D * M],
                    AF.Exp,
                    bias=bias_shift[:],
                    scale=r_all[:, t : t + 1],
                )
                nc.scalar.copy(t_g[:, s, :], raw[:, D * M : DM])

            # Grouped softmax tail
            sums = spool.tile([P, G, D], F32, tag="sums")
            nc.vector.reduce_sum(
                sums[:],
                ex_g[:].rearrange("p s (d m) -> p s d m", d=D, m=M),
                axis=mybir.AxisListType.X,
            )
            prod = spool.tile([P, G, D * M], F32, tag="prod")
            nc.vector.tensor_mul(prod[:], ex_g[:], t_g[:])
            wsum = spool.tile([P, G, D], F32, tag="wsum")
            nc.vector.reduce_sum(
                wsum[:],
                prod[:].rearrange("p s (d m) -> p s d m", d=D, m=M),
                axis=mybir.AxisListType.X,
            )
            rsums = spool.tile([P, G, D], F32, tag="rsums")
            nc.vector.reciprocal(rsums[:], sums[:])
            datt0 = spool.tile([P, G, D], F32, tag="datt0")
            nc.vector.tensor_mul(datt0[:], wsum[:], rsums[:])
            datt = spool.tile([P, G, D], F32, tag="datt")
            rg = r_all[:, g * G : (g + 1) * G]
            nc.vector.tensor_mul(
                datt[:], datt0[:], rg[:, :, None].broadcast_to([P, G, D])
            )
            ex2 = spool.tile([P, G, D], F32, tag="ex2")
            nc.scalar.activation(ex2[:], datt[:], AF.Exp, bias=bias_shift[:])
            sumd = spool.tile([P, G], F32, tag="sumd")
            nc.vector.reduce_sum(sumd[:], ex2[:], axis=mybir.AxisListType.X)
            rd = spool.tile([P, G], F32, tag="rd")
            nc.vector.reciprocal(rd[:], sumd[:])
            nc.vector.tensor_mul(
                out_sb[:, g * G : (g + 1) * G, :],
                ex2[:],
                rd[:, :, None].broadcast_to([P, G, D]),
            )

            out_v = out[:, :].rearrange("(t p) d -> p t d", p=P)
            nc.sync.dma_start(
                out_v[:, g * G : (g + 1) * G, :], out_sb[:, g * G : (g + 1) * G, :]
            )

    # All ACT functions used (Exp, Copy/Identity) live in one table set; steer
    # the table-load placement pass to a single covering set to avoid
    # alternating ~2.7us table loads.
    mine = {AF.Exp, AF.Ln, AF.Square, AF.Copy, AF.Identity}
    orig_tables = bacc.get_activation_tables

    def _patched(arch):
        return {
            name: (fns if name == "natural_log_exp_and_others" else fns - mine)
            for name, fns in orig_tables(arch).items()
        }

    bacc.get_activation_tables = _patched
    try:
        nc.finalize()
    finally:
        bacc.get_activation_tables = orig_tables
    return nc


def _get_nc():
    if "nc" not in _CACHE:
        _CACHE["nc"] = _build_nc()
    return _CACHE["nc"]


def _host_prep(feature, W_topic, W_domain, memory):
    """R matrix, bf16 splits and per-row scale factors, per core."""
    BF = np.float16
    mem_flat = memory.reshape(D * M, E).astype(np.float64)
    Pm = mem_flat @ W_topic.astype(np.float64)
    Qm = mem_flat @ W_domain.astype(np.float64)
    R = np.concatenate([Pm, Qm], axis=0).astype(np.float32)  # [180, 768]
    Rhi = R.astype(BF)
    Rlo = (R - Rhi.astype(np.float32)).astype(BF)
    rt2 = np.concatenate([Rhi.T, Rlo.T], axis=1)  # [768, 360] bf16
    rt2 = np.ascontiguousarray(rt2)

    f = np.asarray(feature, dtype=np.float32)
    norm2 = (f.astype(np.float64) ** 2).sum(axis=1)
    r_rows = (TAU / np.sqrt(norm2)).astype(np.float32)  # [B]

    per_core = []
    for c in range(NCORES):
        fc = f[c * BC : (c + 1) * BC]
        ft = np.ascontiguousarray(fc.T)  # [768, BC] f32
        fhi = ft.astype(BF)
        flo = (ft - fhi.astype(np.float32)).astype(BF)
        rin = np.ascontiguousarray(
            r_rows[c * BC : (c + 1) * BC].reshape(NT, P).T
        )  # [P, NT]
        per_core.append(
            {"fhi": fhi, "flo": flo, "rt2": rt2, "rin": rin}
        )
    return per_core


def kernel(feature, category, W_topic, W_domain, memory):
    from concourse.bass_utils import run_bass_kernel_spmd

    in_maps = _host_prep(
        feature, np.asarray(W_topic), np.asarray(W_domain), np.asarray(memory)
    )
    nc = _get_nc()
    res = run_bass_kernel_spmd(nc, in_maps, core_ids=list(range(NCORES)))
    outs = [res.results[c]["out"] for c in range(NCORES)]
    full = np.concatenate(outs, axis=0)  # [B, 9]
    return full[:, None, :].astype(np.float32)



# revision 3
# speedup vs baseline: 1.3664x; 1.3664x over previous
"""Trainium2 Bass kernel for nn_MemoryNetwork (scatter_memory).

Computation (reference, per batch row b):
    f = feature / ||feature||                       [B, 768]
    topic = f @ W_topic.T ; dom = f @ W_domain.T    [B, 256]
    att   = softmax_m(TAU * topic . memory[d,m])    [B, 9, 10]
    sep   = sum_m att * memory[d,m]                 [B, 9, 256]
    out   = softmax_d(TAU * sep . dom)              [B, 1, 9]

Reformulation: fold the tiny memory banks and TAU into the projections on
the host (f normalized host-side too, so no per-row scale on device):
    RS = TAU * mem_flat @ W_topic   [90, 768]
    RT = TAU * mem_flat @ W_domain  [90, 768]
    rawS = fn @ RS.T ; rawT = fn @ RT.T             [B, 90] each
    ex   = exp(rawS - SHIFT)          (softmax_m numerator; logits in
                                       [-123, 105] so a const shift is safe)
    datt = (sum_m ex*rawT) / (sum_m ex)
    out  = softmax_d(datt)            (const shift again)

Precision: rawS feeds an exponent with TAU-amplified spread, so it needs
~2^-15 relative accuracy on f; rawT only enters linearly and tolerates
plain fp16. Scheme (validated host-side, rel err 7.0e-3 vs 2e-2 gate):
    fhi  = fp16(fn);  flo8 = e4m3((fn - fhi) * 2^17)   (DMA: 3 B/elem)
    RhiS/RloS = fp16 split of RS;  RhiT = fp16(RT);  RS8 = e4m3(RS)
    rawS = fhi@RhiS + fhi@RloS + (flo8@RS8) * 2^-17;  rawT = fhi@RhiT
Per 128-contraction chunk that is 3 PE matmuls: 180-col fp16 [RhiS|RhiT],
90-col fp16 RloS (PSUM-accumulated onto the S columns), 90-col fp8 into a
separate bank (carries the 2^17 scale). LDWEIGHTS fully hides under the
streams (measured), so the PE floor is ~360 cols/chunk = 150 ns warm.

Sharding: data-parallel over B across 8 cores (4096 rows each). All DRAM
layouts are pre-tiled host-side so every DMA descriptor is >=3 KB
contiguous per partition. A burst of dependency-free warmup matmuls at
t=0 starts the PE HAM clock ramp (1.2 -> 2.4 GHz) while DMA fills.
"""

import sys

sys.path.insert(0, "/opt/trn_rl_repo")

import numpy as np

B, IN, E, D, M = 32768, 768, 256, 9, 10
NCORES = 8
BC = B // NCORES  # rows per core
P = 128           # partition tile
NT = BC // P      # batch tiles per core (32)
G = 8             # tiles per softmax group
NG = NT // G      # groups (4)
HB = 4            # tiles per DMA half-block
KC = IN // P      # contraction chunks (6)
DM = D * M        # 90
TAU = 32.0
SHIFT = 50.0
S8 = 2.0 ** 17    # flo8 pre-scale
N_WARM = 26       # HAM warmup matmuls

_CACHE: dict = {}


def _build_nc(repeat=1):
    from contextlib import ExitStack

    import concourse.bacc as bacc
    import concourse.tile as tile
    from concourse import mybir

    F32 = mybir.dt.float32
    F16 = mybir.dt.float16
    F8 = mybir.dt.float8e4
    AF = mybir.ActivationFunctionType
    ALU = mybir.AluOpType

    nc = bacc.Bacc(trn_type="TRN2")
    fhi = nc.dram_tensor("fhi", [P, NT, KC, P], F16, kind="ExternalInput")
    flo8 = nc.dram_tensor("flo8", [P, NT, KC, P], F8, kind="ExternalInput")
    # rt columns: 0:90 RhiS, 90:180 RhiT, 180:270 RloS
    rt = nc.dram_tensor("rt", [P, KC, 3 * DM], F16, kind="ExternalInput")
    rt8 = nc.dram_tensor("rt8", [P, KC, DM], F8, kind="ExternalInput")
    out = nc.dram_tensor("out", [P, NT, D], F32, kind="ExternalOutput")

    with tile.TileContext(nc) as tc, ExitStack() as ctx:
        const = ctx.enter_context(tc.tile_pool(name="const", bufs=1))
        fpool = ctx.enter_context(tc.tile_pool(name="fts", bufs=3))
        lpool = ctx.enter_context(tc.tile_pool(name="lts", bufs=3))
        stg = ctx.enter_context(tc.tile_pool(name="stg", bufs=2))
        gpool = ctx.enter_context(tc.tile_pool(name="grp", bufs=2))
        spool = ctx.enter_context(tc.tile_pool(name="small", bufs=2))
        apool = ctx.enter_context(tc.tile_pool(name="aps", bufs=4, space="PSUM"))
        bpool = ctx.enter_context(tc.tile_pool(name="bps", bufs=3, space="PSUM"))
        wpool = ctx.enter_context(tc.tile_pool(name="wps", bufs=1, space="PSUM"))

        # Constants: scalar (HWDGE) queue so they land before the first
        # flo8 block on the same ring; fhi stream owns the sync ring.
        rt_sb = const.tile([P, KC, 3 * DM], F16)
        nc.scalar.dma_start(rt_sb[:], rt[:, :, :])
        rt8_sb = const.tile([P, KC, DM], F8)
        nc.scalar.dma_start(rt8_sb[:], rt8[:, :, :])
        bias_shift = const.tile([P, 1], F32)
        nc.gpsimd.memset(bias_shift[:], -SHIFT)
        out_sb = const.tile([P, NT, D], F32)

        # HAM warmup: dependency-free matmuls keep the PE busy from t~0 so
        # the 2.4 GHz un-throttle fires while the first feature DMA lands.
        wz = const.tile([P, P], F16)
        nc.gpsimd.memset(wz[:], 0.0)
        wps = wpool.tile([P, DM], F32)
        for _ in range(N_WARM):
            nc.tensor.matmul(wps[:], wz[:], wz[:, 0:DM], start=True, stop=True)

        for g in range(NG * repeat):
            g = g % NG
            ex_g = gpool.tile([P, G, DM], F32, tag="exg")
            t_g = gpool.tile([P, G, DM], F32, tag="tg")

            for h in range(G // HB):
                hb = g * (G // HB) + h
                fhb = fpool.tile([P, HB, KC, P], F16, tag="fhb")
                lhb = lpool.tile([P, HB, KC, P], F8, tag="lhb")
                nc.sync.dma_start(fhb[:], fhi[:, hb * HB : (hb + 1) * HB])
                nc.scalar.dma_start(lhb[:], flo8[:, hb * HB : (hb + 1) * HB])
                rstg = stg.tile([P, HB, DM], F32, tag="rstg")

                for s in range(HB):
                    sg = h * HB + s  # tile index within group
                    A = apool.tile([P, 2 * DM], F32, tag="A")
                    Bp = bpool.tile([P, DM], F32, tag="B")
                    for k in range(KC):
                        # A[:, 0:90] += fhi@RhiS ; A[:, 90:180] += fhi@RhiT
                        nc.tensor.matmul(
                            A[:], fhb[:, s, k, :], rt_sb[:, k, 0 : 2 * DM],
                            start=(k == 0), stop=False,
                        )
                        # A[:, 0:90] += fhi@RloS  (same-column accumulate)
                        nc.tensor.matmul(
                            A[:, 0:DM], fhb[:, s, k, :],
                            rt_sb[:, k, 2 * DM : 3 * DM],
                            start=False, stop=(k == KC - 1),
                            skip_group_check=True,
                        )
                        # Bp += flo8@RS8   (scaled by 2^17)
                        nc.tensor.matmul(
                            Bp[:], lhb[:, s, k, :], rt8_sb[:, k, :],
                            start=(k == 0), stop=(k == KC - 1),
                        )
                    # rawS = Bp * 2^-17 + A_S  (two ops: DVE reads at most
                    # one PSUM operand per instruction)
                    corr = stg.tile([P, DM], F32, tag="corr")
                    nc.vector.tensor_scalar_mul(corr[:], Bp[:], 1.0 / S8)
                    nc.vector.scalar_tensor_tensor(
                        rstg[:, s, :], corr[:], 1.0, A[:, 0:DM],
                        op0=ALU.mult, op1=ALU.add,
                    )
                    # stage rawT to SBUF (frees the PSUM bank)
                    nc.scalar.copy(t_g[:, sg, :], A[:, DM : 2 * DM])

                # exp over the whole half-block in one ACT op
                nc.scalar.activation(
                    ex_g[:, h * HB : (h + 1) * HB, :], rstg[:],
                    AF.Exp, bias=bias_shift[:],
                )

            # Grouped softmax tail
            sums = spool.tile([P, G, D], F32, tag="sums")
            nc.vector.reduce_sum(
                sums[:],
                ex_g[:].rearrange("p s (d m) -> p s d m", d=D, m=M),
                axis=mybir.AxisListType.X,
            )
            prod = spool.tile([P, G, DM], F32, tag="prod")
            nc.vector.tensor_mul(prod[:], ex_g[:], t_g[:])
            wsum = spool.tile([P, G, D], F32, tag="wsum")
            nc.vector.reduce_sum(
                wsum[:],
                prod[:].rearrange("p s (d m) -> p s d m", d=D, m=M),
                axis=mybir.AxisListType.X,
            )
            rsums = spool.tile([P, G, D], F32, tag="rsums")
            nc.vector.reciprocal(rsums[:], sums[:])
            datt = spool.tile([P, G, D], F32, tag="datt")
            nc.vector.tensor_mul(datt[:], wsum[:], rsums[:])
            ex2 = spool.tile([P, G, D], F32, tag="ex2")
            nc.scalar.activation(ex2[:], datt[:], AF.Exp, bias=bias_shift[:])
            sumd = spool.tile([P, G], F32, tag="sumd")
            nc.vector.reduce_sum(sumd[:], ex2[:], axis=mybir.AxisListType.X)
            rd = spool.tile([P, G], F32, tag="rd")
            nc.vector.reciprocal(rd[:], sumd[:])
            nc.vector.tensor_mul(
                out_sb[:, g * G : (g + 1) * G, :],
                ex2[:],
                rd[:, :, None].broadcast_to([P, G, D]),
            )
            # Output rides the gpsimd queue (last group: sync, idle by then)
            eng = nc.sync if g == NG - 1 else nc.gpsimd
            eng.dma_start(
                out[:, g * G : (g + 1) * G], out_sb[:, g * G : (g + 1) * G, :]
            )

    # Keep Exp + Copy in one ACT table set to avoid ~2.7us table swaps.
    mine = {AF.Exp, AF.Ln, AF.Square, AF.Copy, AF.Identity}
    orig_tables = bacc.get_activation_tables

    def _patched(arch):
        return {
            name: (fns if name == "natural_log_exp_and_others" else fns - mine)
            for name, fns in orig_tables(arch).items()
        }

    bacc.get_activation_tables = _patched
    try:
        nc.finalize()
    finally:
        bacc.get_activation_tables = orig_tables
    return nc


def _get_nc():
    if "nc" not in _CACHE:
        _CACHE["nc"] = _build_nc()
    return _CACHE["nc"]


def _host_prep(feature, W_topic, W_domain, memory):
    """Fold memory+TAU into the projections; split planes; pre-tile layouts."""
    import ml_dtypes

    E4 = ml_dtypes.float8_e4m3

    mem_flat = memory.reshape(D * M, E).astype(np.float64)
    RS = TAU * (mem_flat @ W_topic.astype(np.float64))   # [90, 768]
    RT = TAU * (mem_flat @ W_domain.astype(np.float64))  # [90, 768]
    RhiS = RS.astype(np.float16)
    RloS = (RS - RhiS.astype(np.float64)).astype(np.float16)
    RhiT = RT.astype(np.float16)
    rtcat = np.concatenate([RhiS.T, RhiT.T, RloS.T], axis=1)  # [768, 270]
    rt = np.ascontiguousarray(
        rtcat.reshape(KC, P, 3 * DM).transpose(1, 0, 2)
    )  # [P, KC, 270]
    rt8 = np.ascontiguousarray(
        RS.astype(E4).T.reshape(KC, P, DM).transpose(1, 0, 2)
    )  # [P, KC, 90]

    f = np.asarray(feature, dtype=np.float32)
    fn = f / np.sqrt((f.astype(np.float64) ** 2).sum(axis=1, keepdims=True)).astype(
        np.float32
    )

    per_core = []
    for c in range(NCORES):
        fc = fn[c * BC : (c + 1) * BC]  # [4096, 768]
        fhi = fc.astype(np.float16)
        flo = (fc - fhi.astype(np.float32)) * np.float32(S8)
        # [p, t, k, cc] = fc[t*128+cc, k*128+p]
        fhi_t = np.ascontiguousarray(
            fhi.reshape(NT, P, KC, P).transpose(3, 0, 2, 1)
        )
        flo8_t = np.ascontiguousarray(
            flo.astype(E4).reshape(NT, P, KC, P).transpose(3, 0, 2, 1)
        )
        per_core.append({"fhi": fhi_t, "flo8": flo8_t, "rt": rt, "rt8": rt8})
    return per_core


def kernel(feature, category, W_topic, W_domain, memory):
    from concourse.bass_utils import run_bass_kernel_spmd

    in_maps = _host_prep(
        feature, np.asarray(W_topic), np.asarray(W_domain), np.asarray(memory)
    )
    nc = _get_nc()
    res = run_bass_kernel_spmd(nc, in_maps, core_ids=list(range(NCORES)))
    outs = [
        res.results[c]["out"].transpose(1, 0, 2).reshape(BC, D)
        for c in range(NCORES)
    ]
    full = np.concatenate(outs, axis=0)  # [B, 9]
    return full[:, None, :].astype(np.float32)


# revision 6
# speedup vs baseline: 1.5041x; 1.1008x over previous
"""Trainium2 Bass kernel for nn_MemoryNetwork (scatter_memory).

Computation (reference, per batch row b):
    f = feature / ||feature||                       [B, 768]
    topic = f @ W_topic.T ; dom = f @ W_domain.T    [B, 256]
    att   = softmax_m(TAU * topic . memory[d,m])    [B, 9, 10]
    sep   = sum_m att * memory[d,m]                 [B, 9, 256]
    out   = softmax_d(TAU * sep . dom)              [B, 1, 9]

Reformulation: fold the tiny memory banks and TAU into the projections on
the host (f normalized host-side too, so no per-row scale on device):
    RS = TAU * mem_flat @ W_topic   [90, 768]
    RT = TAU * mem_flat @ W_domain  [90, 768]
    rawS = fn @ RS.T ; rawT = fn @ RT.T             [B, 90] each
    ex   = exp(rawS - SHIFT)          (softmax_m numerator; logits in
                                       [-123, 105] so a const shift is safe)
    datt = (sum_m ex*rawT) / (sum_m ex)
    out  = softmax_d(datt)            (const shift again)

Precision: rawS feeds an exponent with TAU-amplified spread, so it needs
~2^-15 relative accuracy on f; rawT only enters linearly and tolerates
plain fp16. Scheme (validated host-side, rel err 7.0e-3 vs 2e-2 gate):
    fhi  = fp16(fn);  flo8 = e4m3((fn - fhi) * 2^17)   (DMA: 3 B/elem)
    RhiS/RloS = fp16 split of RS;  RhiT = fp16(RT);  RS8 = e4m3(RS)
    rawS = fhi@RhiS + fhi@RloS + (flo8@RS8) * 2^-17;  rawT = fhi@RhiT
Per 128-contraction chunk that is 3 PE matmuls: 180-col fp16 [RhiS|RhiT],
90-col fp16 RloS (PSUM-accumulated onto the S columns), 90-col fp8 into a
separate bank (carries the 2^17 scale). LDWEIGHTS fully hides under the
streams (measured), so the PE floor is ~360 cols/chunk = 150 ns warm.

Sharding: data-parallel over B across 8 cores (4096 rows each). All DRAM
layouts are pre-tiled host-side so every DMA descriptor is >=3 KB
contiguous per partition. A burst of dependency-free warmup matmuls at
t=0 starts the PE HAM clock ramp (1.2 -> 2.4 GHz) while DMA fills.
"""

import sys

sys.path.insert(0, "/opt/trn_rl_repo")

import numpy as np

B, IN, E, D, M = 32768, 768, 256, 9, 10
NCORES = 8
BC = B // NCORES  # rows per core
P = 128           # partition tile
NT = BC // P      # batch tiles per core (32)
G = 8             # tiles per softmax group
NG = NT // G      # groups (4)
HB = 4            # tiles per DMA half-block
KC = IN // P      # contraction chunks (6)
DM = D * M        # 90
TAU = 32.0
SHIFT = 50.0
S8 = 2.0 ** 17    # flo8 pre-scale
N_WARM = 48       # HAM warmup matmuls (bridge the DMA fill, trip the ramp)

_CACHE: dict = {}


def _build_nc(repeat=1):
    from contextlib import ExitStack

    import concourse.bacc as bacc
    import concourse.tile as tile
    from concourse import mybir

    F32 = mybir.dt.float32
    F16 = mybir.dt.float16
    F8 = mybir.dt.float8e4
    AF = mybir.ActivationFunctionType
    ALU = mybir.AluOpType

    nc = bacc.Bacc(trn_type="TRN2")
    fhi = nc.dram_tensor("fhi", [P, NT, KC, P], F16, kind="ExternalInput")
    flo8 = nc.dram_tensor("flo8", [P, NT, KC, P], F8, kind="ExternalInput")
    # rt columns: 0:90 RhiS, 90:180 RhiT, 180:270 RloS
    rt = nc.dram_tensor("rt", [P, KC, 3 * DM], F16, kind="ExternalInput")
    rt8 = nc.dram_tensor("rt8", [P, KC, DM], F8, kind="ExternalInput")
    out = nc.dram_tensor("out", [P, NT, D], F32, kind="ExternalOutput")

    with tile.TileContext(nc) as tc, ExitStack() as ctx:
        const = ctx.enter_context(tc.tile_pool(name="const", bufs=1))
        fpool = ctx.enter_context(tc.tile_pool(name="fts", bufs=4))
        lpool = ctx.enter_context(tc.tile_pool(name="lts", bufs=4))
        stg = ctx.enter_context(tc.tile_pool(name="stg", bufs=2))
        gpool = ctx.enter_context(tc.tile_pool(name="grp", bufs=2))
        spool = ctx.enter_context(tc.tile_pool(name="small", bufs=2))
        apool = ctx.enter_context(tc.tile_pool(name="aps", bufs=4, space="PSUM"))
        bpool = ctx.enter_context(tc.tile_pool(name="bps", bufs=2, space="PSUM"))
        wpool = ctx.enter_context(tc.tile_pool(name="wps", bufs=1, space="PSUM"))

        # All input DMA rides the sync (HWDGE) ring: one queue saturates
        # HBM, and keeping DMA issue off the ACT engine (nc.scalar) leaves
        # it free for the copy/exp work. Outputs ride gpsimd.
        rt_sb = const.tile([P, KC, 3 * DM], F16)
        nc.sync.dma_start(rt_sb[:], rt[:, :, :])
        rt8_sb = const.tile([P, KC, DM], F8)
        nc.sync.dma_start(rt8_sb[:], rt8[:, :, :])
        bias_shift = const.tile([P, 1], F32)
        nc.gpsimd.memset(bias_shift[:], -SHIFT)
        out_sb = const.tile([P, NT, D], F32)

        # HAM warmup: dependency-free matmuls keep the PE busy from t~0 so
        # the 2.4 GHz un-throttle fires while the first feature DMA lands.
        wz = const.tile([P, P], F16)
        nc.gpsimd.memset(wz[:], 0.0)
        wps = wpool.tile([P, DM], F32)
        for _ in range(N_WARM):
            nc.tensor.matmul(wps[:], wz[:], wz[:, 0:DM], start=True, stop=True)

        for g in range(NG * repeat):
            g = g % NG
            ex_g = gpool.tile([P, G, DM], F32, tag="exg")
            # staged [rawS_main | rawT] per tile, copied from PSUM by ACT
            as_g = gpool.tile([P, G, 2 * DM], F32, tag="asg")

            for h in range(G // HB):
                hb = g * (G // HB) + h
                fhb = fpool.tile([P, HB, KC, P], F16, tag="fhb")
                lhb = lpool.tile([P, HB, KC, P], F8, tag="lhb")
                nc.sync.dma_start(fhb[:], fhi[:, hb * HB : (hb + 1) * HB])
                nc.sync.dma_start(lhb[:], flo8[:, hb * HB : (hb + 1) * HB])
                rstg = stg.tile([P, HB, DM], F32, tag="rstg")
                # fp8 corrections for all HB tiles share one PSUM bank
                Bp = bpool.tile([P, HB, DM], F32, tag="B")

                for s in range(HB):
                    sg = h * HB + s  # tile index within group
                    A = apool.tile([P, 2 * DM], F32, tag="A")
                    for k in range(KC):
                        # A[:, 0:90] += fhi@RhiS ; A[:, 90:180] += fhi@RhiT
                        nc.tensor.matmul(
                            A[:], fhb[:, s, k, :], rt_sb[:, k, 0 : 2 * DM],
                            start=(k == 0), stop=False,
                        )
                        # A[:, 0:90] += fhi@RloS  (same-column accumulate)
                        nc.tensor.matmul(
                            A[:, 0:DM], fhb[:, s, k, :],
                            rt_sb[:, k, 2 * DM : 3 * DM],
                            start=False, stop=(k == KC - 1),
                            skip_group_check=True,
                        )
                        # Bp[:, s] += flo8@RS8   (scaled by 2^17)
                        nc.tensor.matmul(
                            Bp[:, s, :], lhb[:, s, k, :], rt8_sb[:, k, :],
                            start=(k == 0), stop=(k == KC - 1),
                        )
                    # stage [rawS_main | rawT] to SBUF (frees the PSUM bank)
                    nc.scalar.copy(as_g[:, sg, :], A[:])

                # one fused rawS = Bp * 2^-17 + A_S per half-block
                nc.vector.scalar_tensor_tensor(
                    rstg[:], Bp[:], 1.0 / S8,
                    as_g[:, h * HB : (h + 1) * HB, 0:DM],
                    op0=ALU.mult, op1=ALU.add,
                )
                # exp over the whole half-block in one ACT op
                nc.scalar.activation(
                    ex_g[:, h * HB : (h + 1) * HB, :], rstg[:],
                    AF.Exp, bias=bias_shift[:],
                )

            # Grouped softmax tail
            sums = spool.tile([P, G, D], F32, tag="sums")
            nc.vector.reduce_sum(
                sums[:],
                ex_g[:].rearrange("p s (d m) -> p s d m", d=D, m=M),
                axis=mybir.AxisListType.X,
            )
            prod = spool.tile([P, G, DM], F32, tag="prod")
            nc.vector.tensor_mul(prod[:], ex_g[:], as_g[:, :, DM : 2 * DM])
            wsum = spool.tile([P, G, D], F32, tag="wsum")
            nc.vector.reduce_sum(
                wsum[:],
                prod[:].rearrange("p s (d m) -> p s d m", d=D, m=M),
                axis=mybir.AxisListType.X,
            )
            rsums = spool.tile([P, G, D], F32, tag="rsums")
            nc.vector.reciprocal(rsums[:], sums[:])
            datt = spool.tile([P, G, D], F32, tag="datt")
            nc.vector.tensor_mul(datt[:], wsum[:], rsums[:])
            ex2 = spool.tile([P, G, D], F32, tag="ex2")
            nc.scalar.activation(ex2[:], datt[:], AF.Exp, bias=bias_shift[:])
            sumd = spool.tile([P, G], F32, tag="sumd")
            nc.vector.reduce_sum(sumd[:], ex2[:], axis=mybir.AxisListType.X)
            rd = spool.tile([P, G], F32, tag="rd")
            nc.vector.reciprocal(rd[:], sumd[:])
            nc.vector.tensor_mul(
                out_sb[:, g * G : (g + 1) * G, :],
                ex2[:],
                rd[:, :, None].broadcast_to([P, G, D]),
            )
            # Output rides the gpsimd queue (last group: sync, idle by then)
            eng = nc.sync if g == NG - 1 else nc.gpsimd
            eng.dma_start(
                out[:, g * G : (g + 1) * G], out_sb[:, g * G : (g + 1) * G, :]
            )

    # Keep Exp + Copy in one ACT table set to avoid ~2.7us table swaps.
    mine = {AF.Exp, AF.Ln, AF.Square, AF.Copy, AF.Identity}
    orig_tables = bacc.get_activation_tables

    def _patched(arch):
        return {
            name: (fns if name == "natural_log_exp_and_others" else fns - mine)
            for name, fns in orig_tables(arch).items()
        }

    bacc.get_activation_tables = _patched
    try:
        nc.finalize()
    finally:
        bacc.get_activation_tables = orig_tables
    return nc


def _get_nc():
    if "nc" not in _CACHE:
        _CACHE["nc"] = _build_nc()
    return _CACHE["nc"]


def _host_prep(feature, W_topic, W_domain, memory):
    """Fold memory+TAU into the projections; split planes; pre-tile layouts."""
    import ml_dtypes

    E4 = ml_dtypes.float8_e4m3

    mem_flat = memory.reshape(D * M, E).astype(np.float64)
    RS = TAU * (mem_flat @ W_topic.astype(np.float64))   # [90, 768]
    RT = TAU * (mem_flat @ W_domain.astype(np.float64))  # [90, 768]
    RhiS = RS.astype(np.float16)
    RloS = (RS - RhiS.astype(np.float64)).astype(np.float16)
    RhiT = RT.astype(np.float16)
    rtcat = np.concatenate([RhiS.T, RhiT.T, RloS.T], axis=1)  # [768, 270]
    rt = np.ascontiguousarray(
        rtcat.reshape(KC, P, 3 * DM).transpose(1, 0, 2)
    )  # [P, KC, 270]
    rt8 = np.ascontiguousarray(
        RS.astype(E4).T.reshape(KC, P, DM).transpose(1, 0, 2)
    )  # [P, KC, 90]

    f = np.asarray(feature, dtype=np.float32)
    fn = f / np.sqrt((f.astype(np.float64) ** 2).sum(axis=1, keepdims=True)).astype(
        np.float32
    )

    per_core = []
    for c in range(NCORES):
        fc = fn[c * BC : (c + 1) * BC]  # [4096, 768]
        fhi = fc.astype(np.float16)
        flo = (fc - fhi.astype(np.float32)) * np.float32(S8)
        # [p, t, k, cc] = fc[t*128+cc, k*128+p]
        fhi_t = np.ascontiguousarray(
            fhi.reshape(NT, P, KC, P).transpose(3, 0, 2, 1)
        )
        flo8_t = np.ascontiguousarray(
            flo.astype(E4).reshape(NT, P, KC, P).transpose(3, 0, 2, 1)
        )
        per_core.append({"fhi": fhi_t, "flo8": flo8_t, "rt": rt, "rt8": rt8})
    return per_core


def kernel(feature, category, W_topic, W_domain, memory):
    from concourse.bass_utils import run_bass_kernel_spmd

    in_maps = _host_prep(
        feature, np.asarray(W_topic), np.asarray(W_domain), np.asarray(memory)
    )
    nc = _get_nc()
    res = run_bass_kernel_spmd(nc, in_maps, core_ids=list(range(NCORES)))
    outs = [
        res.results[c]["out"].transpose(1, 0, 2).reshape(BC, D)
        for c in range(NCORES)
    ]
    full = np.concatenate(outs, axis=0)  # [B, 9]
    return full[:, None, :].astype(np.float32)
